# revision 37
# baseline (speedup 1.0000x reference)
"""Trainium2 Bass kernel for nn_HeatEquation1D.

The reference applies a fixed 62x62 Crank-Nicolson step matrix 100 times to
u0[:, 1:-1] via lax.scan, then zero-pads the boundary columns.  Algebraically
that whole scan is a single matmul:

    out = u0 @ W64,   W64[1:63, 1:63] = (step_matrix^100).T,  zero elsewhere

(the zero rows/cols of W64 implement both the dropped boundary inputs and the
zero Dirichlet boundary outputs).  W64 is computed on the host in float64.

The kernel is memory-bound, so all device I/O is bf16 (the correctness
tolerance of 2e-2 has ~10x margin over bf16 quantization noise of ~2e-3):
the host rounds u0 to bf16, the device reads/writes bf16, and the host casts
the result back to float32.  This halves HBM traffic vs f32.

Device kernel (per core, pure data parallel over 8 cores):
  - u shard (65536, 64) bf16 is processed in 32 blocks of 2048 rows.
  - A block is DMA'd as one contiguous (128, 16, 64) tile X: partition p
    holds 16 consecutive rows (2 KiB contiguous per partition).  Input DMAs
    alternate between the two HWDGE queues (SP and Activation) -- a single
    queue caps input issue throughput and costs ~18 us/pass on HW.
  - For each 128-column chunk c (= 2 rows/partition), PE transpose:
      T1[:, c] = X[:, 2c:2c+2, :].T   (128, 128), bf16 into PSUM.
  - One DVE copy PSUM -> SBUF (bf16 packed -> 2x DVE perf mode).
  - matmul with the *transposed chunk as stationary* and a host-built
    BD = block_diag(W64, W64) (128x128 bf16) as the moving operand:
      Y[:, c] = T1s[:, c].T @ BD
    BD's block-diagonal structure applies W64 to each row of the pair and
    the result lands *batch-major* in PSUM (f32) -- no second transpose.
  - PSUM f32 -> SBUF bf16 copy, alternating per block between DVE and the
    Act engine (either alone becomes the bottleneck on HW), then one
    contiguous 256 KiB DMA out on the GPSIMD (SWDGE) queue -- keeping the
    output stream off the input queues avoids head-of-line blocking.

Per-core traffic: 2 x 8.39 MB.  HW-measured (reps-loop wall-clock diff):
~47 us/pass vs a ~41 us pure-DMA floor and 164 us for the staged f32
baseline (3.5x).  Session-to-session absolute numbers vary ~+-20% with
core placement; the config ordering above was stable within sessions.
"""

import numpy as np

BATCH = 524288
NX = 64
N_INNER = NX - 2
NUM_STEPS = 100
N_CORES = 8
ROWS_PER_CORE = BATCH // N_CORES            # 65536
P = 128
ROWS_PER_PART = 16                          # rows per partition per block
ROWS_PER_BLOCK = P * ROWS_PER_PART          # 2048
N_BLOCKS = ROWS_PER_CORE // ROWS_PER_BLOCK  # 32
CHUNKS = (ROWS_PER_PART * NX) // P          # 8 chunks of 128 columns

# Set by callers that want a profile; results object stashed in LAST_RESULTS.
TRACE = False
LAST_RESULTS = None

_NC_CACHE = {}


def _build_nc(
    reps=1,
    dma_only=False,
    psum_t_f32=False,
    xin_bufs=8,
    t1s_bufs=6,
    yout_bufs=8,
    pst_bufs=3,
    psy_bufs=2,
    copy2_engine="alt",
    copy1_engine="vector",
    out_dma_engine="gpsimd",
    pipeline="pe_transpose",
    in_dma_engines=("sync", "scalar"),
    out_dma_engines=None,
    out_pair=False,
    out_delay=0,
    in_pair=False,
    dma_single_packet=False,
):
    from concourse import bacc, mybir
    from concourse.tile import TileContext

    nc = bacc.Bacc("TRN2", target_bir_lowering=False, debug=False)
    bf16 = mybir.dt.bfloat16
    f32 = mybir.dt.float32

    u = nc.dram_tensor("u", [ROWS_PER_CORE, NX], bf16, kind="ExternalInput")
    bd_d = nc.dram_tensor("bd", [P, P], bf16, kind="ExternalInput")
    id_d = nc.dram_tensor("ident", [P, P], bf16, kind="ExternalInput")
    out = nc.dram_tensor("out", [ROWS_PER_CORE, NX], bf16, kind="ExternalOutput")

    u_r = u.rearrange("(nb p r) f -> nb p r f", p=P, r=ROWS_PER_PART)
    out_r = out.rearrange("(nb p r) f -> nb p r f", p=P, r=ROWS_PER_PART)
    # paired-output view: one DMA per two blocks, partition p holds both
    # blocks' rows 16p..16p+15 (two 2KB-contiguous runs per partition)
    out_p2 = out.rearrange(
        "(np a p r) f -> np p a r f", a=2, p=P, r=ROWS_PER_PART
    )
    u_p2 = u.rearrange(
        "(np a p r) f -> np p a r f", a=2, p=P, r=ROWS_PER_PART
    )
    # row-pair view for the XBAR transpose DMA: block nb as (1024 pairs, 128)
    JP = ROWS_PER_BLOCK // 2                  # 1024 row-pairs per block
    KS = JP // P                              # 8 = pairs per output partition
    u_t = u.rearrange("(nb j two) f -> nb j (two f)", j=JP, two=2)

    t_dt = f32 if psum_t_f32 else bf16
    if out_dma_engines is None:
        out_dma_engines = (out_dma_engine,)

    with TileContext(nc) as tc:
        with (
            tc.tile_pool(name="consts", bufs=1) as cpool,
            tc.tile_pool(name="xin", bufs=xin_bufs) as xpool,
            tc.tile_pool(name="t1s", bufs=t1s_bufs) as tpool,
            tc.tile_pool(name="yout", bufs=yout_bufs) as ypool,
            tc.tile_pool(name="ps_t", bufs=pst_bufs, space="PSUM") as pst,
            tc.tile_pool(name="ps_y", bufs=psy_bufs, space="PSUM") as psy,
        ):
            bd_s = cpool.tile([P, P], bf16)
            id_s = cpool.tile([P, P], bf16)
            nc.sync.dma_start(out=bd_s[:], in_=bd_d[:])
            nc.sync.dma_start(out=id_s[:], in_=id_d[:])

            def eng_copy(engine, out_ap, in_ap):
                if engine == "scalar":
                    nc.scalar.copy(out=out_ap, in_=in_ap)
                elif engine == "vector":
                    nc.vector.tensor_copy(out=out_ap, in_=in_ap)
                elif engine == "gpsimd":
                    nc.gpsimd.tensor_copy(out=out_ap, in_=in_ap)
                else:
                    raise ValueError(engine)

            def copy_out(ys, yp, nb=0):
                if copy2_engine == "alt":
                    eng_copy("vector" if nb % 2 == 0 else "scalar", ys[:], yp[:])
                elif copy2_engine == "alt32":
                    eng_copy("vector" if nb % 5 < 3 else "scalar", ys[:], yp[:])
                elif copy2_engine == "split":
                    half_r = ROWS_PER_PART // 2
                    nc.scalar.copy(out=ys[:, :half_r], in_=yp[:, : CHUNKS // 2])
                    nc.vector.tensor_copy(
                        out=ys[:, half_r:], in_=yp[:, CHUNKS // 2 :]
                    )
                else:
                    eng_copy(copy2_engine, ys[:], yp[:])

            def body_dma_t():
                # XBAR-transpose pipeline: one transpose DMA loads the block
                # directly in (feature-pair, row-pair) layout; matmul k uses
                # the strided stationary slice j=8*Pidx+k so the PSUM result
                # comes out with 16 consecutive rows per partition, keeping
                # the 2KB-contiguous output DMA.
                for nb in range(N_BLOCKS):
                    t1s = tpool.tile([P, P, KS], bf16)
                    eng = in_dma_engines[nb % len(in_dma_engines)]
                    getattr(nc, eng).dma_start(
                        out=t1s[:], in_=u_t[nb], transpose=True
                    )

                    yp = psy.tile([P, KS, 2, NX], f32)
                    for k in range(KS):
                        nc.tensor.matmul(
                            yp[:, k], t1s[:, :, k], bd_s[:], start=True, stop=True
                        )
                    ys = ypool.tile([P, ROWS_PER_PART, NX], bf16)
                    copy_out(ys, yp, nb)
                    o_eng = out_dma_engines[nb % len(out_dma_engines)]
                    getattr(nc, o_eng).dma_start(out=out_r[nb], in_=ys[:])

            def body():
                if pipeline == "dma_transpose":
                    body_dma_t()
                    return
                for nb in range(N_BLOCKS):
                    if in_pair:
                        # one 512KB DMA covers two blocks; partition p holds
                        # rows 16p..16p+15 of each block (two 2KB runs)
                        if nb % 2 == 0:
                            x2 = xpool.tile([P, 2, ROWS_PER_PART, NX], bf16)
                            pair_tiles[1] = x2
                            i_eng = in_dma_engines[
                                (nb // 2) % len(in_dma_engines)
                            ]
                            getattr(nc, i_eng).dma_start(
                                out=x2[:], in_=u_p2[nb // 2]
                            )
                        x = pair_tiles[1][:, nb % 2]
                    else:
                        x = xpool.tile([P, ROWS_PER_PART, NX], bf16)
                        i_eng = in_dma_engines[nb % len(in_dma_engines)]
                        getattr(nc, i_eng).dma_start(out=x[:], in_=u_r[nb])

                    if dma_only:
                        nc.sync.dma_start(out=out_r[nb], in_=x[:])
                        continue

                    t1p = pst.tile([P, CHUNKS, P], t_dt)
                    for c in range(CHUNKS):
                        nc.tensor.transpose(
                            t1p[:, c], x[:, 2 * c : 2 * c + 2, :], id_s[:]
                        )
                    t1s = tpool.tile([P, CHUNKS, P], bf16)
                    eng_copy(copy1_engine, t1s[:], t1p[:])

                    yp = psy.tile([P, CHUNKS, P], f32)
                    for c in range(CHUNKS):
                        nc.tensor.matmul(
                            yp[:, c], t1s[:, c], bd_s[:], start=True, stop=True
                        )
                    if out_pair:
                        if nb % 2 == 0:
                            ys2 = ypool.tile([P, 2, ROWS_PER_PART, NX], bf16)
                            pair_tiles[0] = ys2
                        ys2 = pair_tiles[0]
                        copy_out(ys2[:, nb % 2], yp, nb)
                        if nb % 2 == 1:
                            o_eng = out_dma_engines[
                                (nb // 2) % len(out_dma_engines)
                            ]
                            getattr(nc, o_eng).dma_start(
                                out=out_p2[nb // 2], in_=ys2[:]
                            )
                    else:
                        ys = ypool.tile([P, ROWS_PER_PART, NX], bf16)
                        copy_out(ys, yp, nb)
                        pending.append((nb, ys))
                        if len(pending) > out_delay:
                            flush_one()
                for _ in range(len(pending)):
                    flush_one()

            pair_tiles = [None, None]
            pending = []

            def flush_one():
                onb, oys = pending.pop(0)
                o_eng = out_dma_engines[onb % len(out_dma_engines)]
                getattr(nc, o_eng).dma_start(out=out_r[onb], in_=oys[:])

            if reps > 1:
                with tc.For_i(
                    0, reps, 1, hint_engines=(mybir.EngineType.PE,)
                ) as _i:
                    body()
            else:
                body()

    nc.compile()
    return nc


def _host_matrices(step_matrix):
    import ml_dtypes

    m = np.asarray(step_matrix, dtype=np.float64)
    w_inner = np.linalg.matrix_power(m, NUM_STEPS).T  # right-multiplier, f64
    w64 = np.zeros((NX, NX), dtype=np.float64)
    w64[1 : NX - 1, 1 : NX - 1] = w_inner
    bd = np.zeros((P, P), dtype=np.float64)
    bd[:NX, :NX] = w64
    bd[NX:, NX:] = w64
    return bd.astype(ml_dtypes.bfloat16)


def kernel(u0, step_matrix):
    global LAST_RESULTS
    import ml_dtypes

    from concourse.bass_utils import run_bass_kernel_spmd

    u0 = np.asarray(u0)
    assert u0.shape == (BATCH, NX), u0.shape
    u0_bf16 = np.ascontiguousarray(u0).astype(ml_dtypes.bfloat16)

    bd = _host_matrices(step_matrix)
    ident = np.eye(P, dtype=ml_dtypes.bfloat16)

    if "nc" not in _NC_CACHE:
        _NC_CACHE["nc"] = _build_nc()
    nc = _NC_CACHE["nc"]

    shards = np.split(u0_bf16, N_CORES, axis=0)
    in_maps = [{"u": s, "bd": bd, "ident": ident} for s in shards]
    res = run_bass_kernel_spmd(
        nc, in_maps, core_ids=list(range(N_CORES)), trace=TRACE
    )
    LAST_RESULTS = res
    return np.concatenate([r["out"] for r in res.results], axis=0).astype(
        np.float32
    )
